# revision 16
# baseline (speedup 1.0000x reference)
"""Trainium2 Bass kernel: pre-LN + 16-head attention (b=2, n=2048, d=1024) + out-proj.

Sharding over 8 NeuronCores: core c handles batch c//4 and heads 4*(c%4) .. +4
(data parallel over batch x tensor parallel over heads).  Each core returns a
partial out-projection [2048, 1024]; the host sums the 4 head-group partials
per batch and adds b_out (the unshard of the reduction-sharded output).

Device algorithm per core (T=2048 tokens, 4 heads, d=64):
  - x is fed pre-transposed (x^T, [1024, T]) so every matmul has its
    contraction dim on partitions; LayerNorm commutes with the projection:
      qkv^T[f,t] = A[t]*( (x @ W')^T[f,t] + (-mu[t])*colsum(W')[f]
                          + (1/A[t])*(beta@W)[f] )
    with W' = diag(gamma) @ W, A = rsqrt(var+eps).  The two rank-1 terms are
    single-partition matmul accumulates; the A[t] factor is applied at PSUM
    eviction (DVE multiply by a broadcast row / ACT per-partition scale).
  - Scores are built transposed (s^T[k,q] = K^T.T @ Q^T) so softmax's exp is a
    plain ACT pass and P@V needs no transposes: attnU^T = [V|1].T @ exp(s^T),
    whose 65th row is the softmax denominator.
  - Normalization is a broadcast (selector-matmul) reciprocal multiply; the
    out-projection consumes attn^T directly as the stationary operand.
"""

import os
import sys

for _p in ("/opt/trn_rl_repo", "/root/.axon_site/_ro/trn_rl_repo"):
    if os.path.isdir(_p) and _p not in sys.path:
        sys.path.append(_p)

import ml_dtypes
import numpy as np

import concourse.mybir as mybir
import concourse.tile as tile
from concourse import bacc
from concourse import bass_utils as _bu
from concourse.bass_utils import run_bass_kernel_spmd

if os.environ.get("KERNEL_LDWOPT", "0") == "1" and not getattr(_bu, "_ldw_patched", False):
    _orig_run_command = _bu.run_command

    def _patched_run_command(argv, **kwargs):
        argv = ["--enable-ldw-opt=true" if a == "--enable-ldw-opt=false" else a
                for a in argv]
        return _orig_run_command(argv, **kwargs)

    _bu.run_command = _patched_run_command
    _bu._ldw_patched = True

F32 = mybir.dt.float32
F32R = mybir.dt.float32r
BF16 = mybir.dt.bfloat16
AF = mybir.ActivationFunctionType
ALU = mybir.AluOpType

T = 2048          # tokens per core (one batch element)
C = 1024          # model dim
NH = 4            # heads per core
D = 64            # head dim
FQ = NH * D       # 256 per-core q/k/v feature cols
NCT = C // 128    # 8 contraction tiles
NTT = T // 128    # 16 token tiles
QC = 512          # q-chunk width
NQC = T // QC     # 4 q-chunks
EPS = 1e-5

LAST_RESULT = None
_CACHE = {}


def _emit_proj(tc, nc, dram, cpool, state):
    """Phases A-C: stats, Q^T/K^T projections, V-natural projection."""
    (xt_d, wq_d, wk_d, wv_d, sv_d, bv_d, a_d) = dram
    negmu, recipa, abc, a_col, q2t, k2t, vna = state
    ones128 = cpool.tile([128, 1], BF16, tag="ones128")
    nc.vector.memset(ones128[:], 1.0)
    ones1 = cpool.tile([1, 128], F32, tag="ones1")
    nc.vector.memset(ones1[:], 1.0)
    srows = [cpool.tile([1, FQ], BF16, tag=f"srow{i}", name=f"srow{i}")
             for i in range(3)]
    brows = [cpool.tile([1, FQ], BF16, tag=f"brow{i}", name=f"brow{i}")
             for i in range(3)]
    for i in range(3):
        nc.sync.dma_start(srows[i][:], sv_d[i:i + 1, :])
        nc.sync.dma_start(brows[i][:], bv_d[i:i + 1, :])

    with (
        tc.tile_pool(name="xt", bufs=1) as xtpool,
        tc.tile_pool(name="w", bufs=1) as wpool,
        tc.tile_pool(name="sq", bufs=2) as sqpool,
        tc.tile_pool(name="small", bufs=2) as smpool,
        tc.tile_pool(name="psA", bufs=1, space="PSUM") as psA,
        tc.tile_pool(name="psB", bufs=2, space="PSUM") as psB,
    ):
        xts = []
        for ci in range(NCT):
            xt_sb = xtpool.tile([128, T], BF16, tag=f"xt{ci}", name=f"xt{ci}")
            nc.sync.dma_start(xt_sb[:], xt_d[ci * 128:(ci + 1) * 128, :])
            xts.append(xt_sb)

        wqs, wks, wvs = [], [], []
        for ci in range(NCT):
            for lst, src, nm in ((wqs, wq_d, "wq"), (wks, wk_d, "wk"),
                                 (wvs, wv_d, "wv")):
                w_sb = wpool.tile([128, FQ], BF16, tag=f"{nm}{ci}",
                                  name=f"{nm}{ci}")
                nc.sync.dma_start(w_sb[:], src[ci * 128:(ci + 1) * 128, :])
                lst.append(w_sb)

        # ---- stats: sum(x) and sum(x^2) over c via ones-matmuls ----
        for tch in range(NQC):
            ts = slice(tch * QC, (tch + 1) * QC)
            ps_sum = psA.tile([1, QC], F32, tag="ps_sum")
            ps_ssq = psA.tile([1, QC], F32, tag="ps_ssq")
            for ci in range(NCT):
                nc.tensor.matmul(ps_sum[:], ones128[:], xts[ci][:, ts],
                                 start=(ci == 0), stop=(ci == NCT - 1))
                xsq = sqpool.tile([128, QC], BF16, tag="xsq")
                nc.vector.tensor_mul(xsq[:], xts[ci][:, ts], xts[ci][:, ts])
                nc.tensor.matmul(ps_ssq[:], ones128[:], xsq[:],
                                 start=(ci == 0), stop=(ci == NCT - 1))
            nc.scalar.activation(negmu[0:1, ts], ps_sum[:], AF.Copy,
                                 scale=-1.0 / C)
            mu2 = smpool.tile([1, QC], F32, tag="mu2")
            nc.vector.tensor_mul(mu2[:], negmu[0:1, ts], negmu[0:1, ts])
            mu2me = smpool.tile([1, QC], F32, tag="mu2me")
            nc.vector.tensor_scalar_add(mu2me[:], mu2[:], -EPS)
            vare = smpool.tile([1, QC], F32, tag="vare")
            nc.vector.scalar_tensor_tensor(vare[:], ps_ssq[:], 1.0 / C,
                                           mu2me[:], ALU.mult, ALU.subtract)
            nc.scalar.activation(recipa[0:1, ts], vare[:], AF.Sqrt)
            rvar = smpool.tile([1, QC], F32, tag="rvar")
            nc.vector.reciprocal(rvar[:], vare[:])
            a_row = smpool.tile([1, QC], F32, tag="a_row")
            nc.scalar.activation(a_row[:], rvar[:], AF.Sqrt)
            # broadcast A to 128 partitions
            ps_abc = psB.tile([128, QC], F32, tag="ps_v")
            nc.tensor.matmul(ps_abc[:], ones1[:], a_row[:],
                             start=True, stop=True)
            nc.scalar.activation(abc[:, ts], ps_abc[:], AF.Copy)
            nc.sync.dma_start(a_d[tch * QC:(tch + 1) * QC, 0:1], a_row[0:1, :])
        # A as per-t-tile columns via DRAM round-trip
        for ti in range(NTT):
            nc.sync.dma_start(a_col[:, ti:ti + 1],
                              a_d[ti * 128:(ti + 1) * 128, 0:1])

        # ---- Q^T / K^T f-tiles ----
        for p in range(2):
            for wlist, dst, srow, brow in (
                (wqs, q2t[p], srows[0], brows[0]),
                (wks, k2t[p], srows[1], brows[1]),
            ):
                fsl = slice(p * 128, (p + 1) * 128)
                for tch in range(NQC):
                    ts = slice(tch * QC, (tch + 1) * QC)
                    ps = psB.tile([128, QC], F32, tag="ps_qk")
                    for ci in range(NCT):
                        nc.tensor.matmul(ps[:], wlist[ci][:, fsl],
                                         xts[ci][:, ts], start=(ci == 0),
                                         stop=False)
                    nc.tensor.matmul(ps[:], srow[0:1, fsl], negmu[0:1, ts],
                                     start=False, stop=False)
                    nc.tensor.matmul(ps[:], brow[0:1, fsl], recipa[0:1, ts],
                                     start=False, stop=True)
                    nc.vector.tensor_mul(dst[:, ts], ps[:], abc[:, ts])

        # ---- V natural [t, f] with ones columns ----
        for ti in range(NTT):
            tsl = slice(ti * 128, (ti + 1) * 128)
            ps = psB.tile([128, NH, D], F32, tag="ps_v")
            ps2 = ps.rearrange("p a b -> p (a b)")
            for ci in range(NCT):
                nc.tensor.matmul(ps2, xts[ci][:, tsl], wvs[ci][:],
                                 start=(ci == 0), stop=False)
            nc.tensor.matmul(ps2, negmu[0:1, tsl], srows[2][:],
                             start=False, stop=False)
            nc.tensor.matmul(ps2, recipa[0:1, tsl], brows[2][:],
                             start=False, stop=True)
            nc.vector.tensor_scalar_mul(vna[ti][:, :, 0:D], ps[:],
                                        a_col[:, ti:ti + 1])
            nc.vector.memset(vna[ti][:, :, D:D + 1], 1.0)


def _emit_attention(tc, nc, q2t, k2t, vna, esel, attnu, denom, recips):
    QW = QC
    with (
        tc.tile_pool(name="exps", bufs=3) as epool,
        tc.tile_pool(name="spill", bufs=2) as spool,
        tc.tile_pool(name="ps_s", bufs=2, space="PSUM") as ps_s_pool,
        tc.tile_pool(name="ps_pv", bufs=1, space="PSUM") as ps_pv_pool,
    ):
        for qc in range(T // QW):
            qs = slice(qc * QW, (qc + 1) * QW)
            for p in range(2):
                ps_pv_a = ps_pv_pool.tile([D + 1, QW], F32, tag="pv_a")
                ps_pv_b = ps_pv_pool.tile([D + 1, QW], F32, tag="pv_b")
                for kt in range(NTT):
                    ksl = slice(kt * 128, (kt + 1) * 128)
                    ps_s2 = ps_s_pool.tile([128, 2 * QW], F32, tag="s2")
                    nc.tensor.matmul(ps_s2[:, 0:QW], k2t[p][0:D, ksl],
                                     q2t[p][0:D, qs], start=True, stop=True)
                    nc.tensor.matmul(ps_s2[:, QW:2 * QW], k2t[p][D:2 * D, ksl],
                                     q2t[p][D:2 * D, qs], start=True,
                                     stop=True)
                    es2 = epool.tile([128, 2 * QW], BF16, tag="es2")
                    nc.scalar.activation(es2[:], ps_s2[:], AF.Exp,
                                         scale=D ** -0.5)
                    nc.tensor.matmul(ps_pv_a[:], vna[kt][:, 2 * p, :],
                                     es2[:, 0:QW], start=(kt == 0),
                                     stop=(kt == NTT - 1))
                    nc.tensor.matmul(ps_pv_b[:], vna[kt][:, 2 * p + 1, :],
                                     es2[:, QW:2 * QW], start=(kt == 0),
                                     stop=(kt == NTT - 1))
                # evict: aligned DVE copies to SBUF, then SBUF->SBUF DMA for
                # the partition-shifted half and the denom rows
                nc.vector.tensor_copy(attnu[p][0:D, qs], ps_pv_a[0:D, :])
                tmpr = spool.tile([D + 1, QW], BF16, tag="tmpr")
                nc.vector.tensor_copy(tmpr[D:D + 1, :], ps_pv_a[D:D + 1, :])
                nc.sync.dma_start(denom[2 * p:2 * p + 1, qs],
                                  tmpr[D:D + 1, :])
                tmpb = spool.tile([D + 1, QW], BF16, tag="tmpb")
                nc.vector.tensor_copy(tmpb[:], ps_pv_b[:])
                nc.sync.dma_start(attnu[p][D:2 * D, qs], tmpb[0:D, :])
                nc.sync.dma_start(denom[2 * p + 1:2 * p + 2, qs],
                                  tmpb[D:D + 1, :])
            with nc.allow_low_precision(reason="f32r stores full fp32 bits"):
                nc.vector.reciprocal(recips[:, qs], denom[:, qs])
            for p in range(2):
                ps_r = ps_s_pool.tile([128, QW], F32, tag="s2")
                nc.tensor.matmul(ps_r[:], esel[:, p * 128:(p + 1) * 128],
                                 recips[:, qs], start=True, stop=True)
                nc.vector.tensor_mul(attnu[p][:, qs], attnu[p][:, qs],
                                     ps_r[:])


def _emit_outproj(tc, nc, wo_d, out_d, attnu):
    with (
        tc.tile_pool(name="wo2", bufs=1) as wo2pool,
        tc.tile_pool(name="osb", bufs=3) as opool,
        tc.tile_pool(name="ps_o", bufs=3, space="PSUM") as ps_o_pool,
    ):
        wos2 = []
        for p in range(2):
            wo_sb = wo2pool.tile([128, C], BF16, tag=f"wo2_{p}",
                                 name=f"wo2_{p}")
            nc.sync.dma_start(wo_sb[:], wo_d[p * 128:(p + 1) * 128, :])
            wos2.append(wo_sb)
        for ti in range(NTT):
            tsl = slice(ti * 128, (ti + 1) * 128)
            for oc in range(2):
                osl = slice(oc * QC, (oc + 1) * QC)
                ps = ps_o_pool.tile([128, QC], F32, tag="ps_o")
                nc.tensor.matmul(ps[:], attnu[0][:, tsl], wos2[0][:, osl],
                                 start=True, stop=False)
                nc.tensor.matmul(ps[:], attnu[1][:, tsl], wos2[1][:, osl],
                                 start=False, stop=True)
                o_sb = opool.tile([128, QC], F32, tag="o_sb")
                nc.vector.tensor_copy(o_sb[:], ps[:])
                nc.sync.dma_start(out_d[tsl, osl], o_sb[:])


def _emit(tc):
    nc = tc.nc
    xt_d = nc.dram_tensor("xt", [C, T], BF16, kind="ExternalInput").ap()
    wq_d = nc.dram_tensor("wq", [C, FQ], BF16, kind="ExternalInput").ap()
    wk_d = nc.dram_tensor("wk", [C, FQ], BF16, kind="ExternalInput").ap()
    wv_d = nc.dram_tensor("wv", [C, FQ], BF16, kind="ExternalInput").ap()
    wo_d = nc.dram_tensor("wo", [FQ, C], BF16, kind="ExternalInput").ap()
    sv_d = nc.dram_tensor("svecs", [3, FQ], BF16, kind="ExternalInput").ap()
    bv_d = nc.dram_tensor("bvecs", [3, FQ], BF16, kind="ExternalInput").ap()
    es_d = nc.dram_tensor("esel", [NH, FQ], F32R, kind="ExternalInput").ap()
    out_d = nc.dram_tensor("out", [T, C], F32, kind="ExternalOutput").ap()
    a_d = nc.dram_tensor("a_scratch", [T, 1], F32, kind="Internal").ap()

    with (
        tc.tile_pool(name="const", bufs=1) as cpool,
        tc.tile_pool(name="persist", bufs=1) as ppool,
    ):
        esel = cpool.tile([NH, FQ], F32R, tag="esel")
        nc.sync.dma_start(esel[:], es_d[:])

        negmu = ppool.tile([1, T], BF16, tag="negmu")
        recipa = ppool.tile([1, T], BF16, tag="recipa")   # sqrt(var+eps) = 1/A
        abc = ppool.tile([128, T], F32R, tag="abc")       # A bcast to 128 parts
        a_col = ppool.tile([128, NTT], F32, tag="a_col")
        q2t = [ppool.tile([128, T], BF16, tag=f"q2t{p}", name=f"q2t{p}")
               for p in range(2)]
        k2t = [ppool.tile([128, T], BF16, tag=f"k2t{p}", name=f"k2t{p}")
               for p in range(2)]
        vna = [ppool.tile([128, NH, D + 1], BF16, tag=f"vna{i}",
                          name=f"vna{i}") for i in range(NTT)]

        _emit_proj(tc, nc, (xt_d, wq_d, wk_d, wv_d, sv_d, bv_d, a_d),
                   cpool, (negmu, recipa, abc, a_col, q2t, k2t, vna))

        with tc.tile_pool(name="late", bufs=1) as lpool:
            attnu = [lpool.tile([128, T], BF16, tag=f"attnu{p}",
                                name=f"attnu{p}") for p in range(2)]
            denom = lpool.tile([NH, T], BF16, tag="denom")
            recips = lpool.tile([NH, T], F32R, tag="recips")
            _emit_attention(tc, nc, q2t, k2t, vna, esel, attnu, denom, recips)
            _emit_outproj(tc, nc, wo_d, out_d, attnu)


def _build():
    if "nc" in _CACHE:
        return _CACHE["nc"]
    nc = bacc.Bacc("TRN2", target_bir_lowering=False, debug=False,
                   enable_asserts=False)
    with tile.TileContext(nc) as tc:
        _emit(tc)
    nc.compile()
    _CACHE["nc"] = nc
    return nc


def kernel(x, gamma, beta, w_qkv, w_out, b_out):
    global LAST_RESULT
    x = np.asarray(x, np.float32)
    gamma = np.asarray(gamma, np.float32)
    beta = np.asarray(beta, np.float32)
    w_qkv = np.asarray(w_qkv, np.float32)
    w_out = np.asarray(w_out, np.float32)
    b_out = np.asarray(b_out, np.float32)

    nc = _build()

    wq_full = gamma[:, None] * w_qkv[:, 0:1024]
    wk_full = gamma[:, None] * w_qkv[:, 1024:2048]
    wv_full = gamma[:, None] * w_qkv[:, 2048:3072]
    bq_full = beta @ w_qkv[:, 0:1024]
    bk_full = beta @ w_qkv[:, 1024:2048]
    bv_full = beta @ w_qkv[:, 2048:3072]

    xts = [np.ascontiguousarray(x[b].T) for b in range(2)]
    esel = np.zeros((NH, FQ), np.float32)
    for h in range(NH):
        esel[h, h * D:(h + 1) * D] = 1.0

    in_maps = []
    for c in range(8):
        b, g = divmod(c, 4)
        fsl = slice(g * FQ, (g + 1) * FQ)
        wq = np.ascontiguousarray(wq_full[:, fsl])
        wk = np.ascontiguousarray(wk_full[:, fsl])
        wv = np.ascontiguousarray(wv_full[:, fsl])
        svecs = np.stack([wq.sum(0), wk.sum(0), wv.sum(0)])
        bvecs = np.stack([bq_full[fsl], bk_full[fsl], bv_full[fsl]])
        bf = ml_dtypes.bfloat16
        in_maps.append({
            "xt": xts[b].astype(bf),
            "wq": wq.astype(bf), "wk": wk.astype(bf), "wv": wv.astype(bf),
            "wo": np.ascontiguousarray(w_out[fsl, :]).astype(bf),
            "svecs": np.ascontiguousarray(svecs).astype(bf),
            "bvecs": np.ascontiguousarray(bvecs).astype(bf),
            "esel": esel,
        })

    trace = bool(int(os.environ.get("KERNEL_TRACE", "0")))
    trace_cores = None
    if trace:
        tc_env = os.environ.get("KERNEL_TRACE_CORES", "0")
        trace_cores = [int(v) for v in tc_env.split(",")]
    res = run_bass_kernel_spmd(nc, in_maps, core_ids=list(range(8)),
                               trace=trace, trace_cores=trace_cores)
    LAST_RESULT = res

    parts = [res.results[c]["out"] for c in range(8)]
    out = np.stack([
        parts[0] + parts[1] + parts[2] + parts[3],
        parts[4] + parts[5] + parts[6] + parts[7],
    ])
    return (out + b_out).astype(np.float32)


# revision 17
# speedup vs baseline: 1.0300x; 1.0300x over previous
"""Trainium2 Bass kernel: pre-LN + 16-head attention (b=2, n=2048, d=1024) + out-proj.

Sharding over 8 NeuronCores: core c handles batch c//4 and heads 4*(c%4) .. +4
(data parallel over batch x tensor parallel over heads).  Each core returns a
partial out-projection [2048, 1024]; the host sums the 4 head-group partials
per batch and adds b_out (the unshard of the reduction-sharded output).

Device algorithm per core (T=2048 tokens, 4 heads, d=64):
  - x is fed pre-transposed (x^T, [1024, T]) so every matmul has its
    contraction dim on partitions; LayerNorm commutes with the projection:
      qkv^T[f,t] = A[t]*( (x @ W')^T[f,t] + (-mu[t])*colsum(W')[f]
                          + (1/A[t])*(beta@W)[f] )
    with W' = diag(gamma) @ W, A = rsqrt(var+eps).  The two rank-1 terms are
    single-partition matmul accumulates; the A[t] factor is applied at PSUM
    eviction (DVE multiply by a broadcast row / ACT per-partition scale).
  - Scores are built transposed (s^T[k,q] = K^T.T @ Q^T) so softmax's exp is a
    plain ACT pass and P@V needs no transposes: attnU^T = [V|1].T @ exp(s^T),
    whose 65th row is the softmax denominator.
  - Normalization is a broadcast (selector-matmul) reciprocal multiply; the
    out-projection consumes attn^T directly as the stationary operand.
"""

import os
import sys

for _p in ("/opt/trn_rl_repo", "/root/.axon_site/_ro/trn_rl_repo"):
    if os.path.isdir(_p) and _p not in sys.path:
        sys.path.append(_p)

import ml_dtypes
import numpy as np

import concourse.mybir as mybir
import concourse.tile as tile
from concourse import bacc
from concourse import bass_utils as _bu
from concourse.bass_utils import run_bass_kernel_spmd

if os.environ.get("KERNEL_LDWOPT", "0") == "1" and not getattr(_bu, "_ldw_patched", False):
    _orig_run_command = _bu.run_command

    def _patched_run_command(argv, **kwargs):
        argv = ["--enable-ldw-opt=true" if a == "--enable-ldw-opt=false" else a
                for a in argv]
        return _orig_run_command(argv, **kwargs)

    _bu.run_command = _patched_run_command
    _bu._ldw_patched = True

F32 = mybir.dt.float32
F32R = mybir.dt.float32r
BF16 = mybir.dt.bfloat16
AF = mybir.ActivationFunctionType
ALU = mybir.AluOpType

T = 2048          # tokens per core (one batch element)
C = 1024          # model dim
NH = 4            # heads per core
D = 64            # head dim
FQ = NH * D       # 256 per-core q/k/v feature cols
NCT = C // 128    # 8 contraction tiles
NTT = T // 128    # 16 token tiles
QC = 512          # q-chunk width
NQC = T // QC     # 4 q-chunks
EPS = 1e-5

LAST_RESULT = None
_CACHE = {}


def _emit_proj(tc, nc, dram, cpool, state):
    """Phases A-C: stats, Q^T/K^T projections, V-natural projection."""
    (xt_d, wq_d, wk_d, wv_d, sv_d, bv_d, a_d) = dram
    negmu, recipa, abc, a_col, q2t, k2t, vna = state
    ones128 = cpool.tile([128, 1], BF16, tag="ones128")
    nc.vector.memset(ones128[:], 1.0)
    ones1 = cpool.tile([1, 128], F32, tag="ones1")
    nc.vector.memset(ones1[:], 1.0)
    srows = [cpool.tile([1, FQ], BF16, tag=f"srow{i}", name=f"srow{i}")
             for i in range(3)]
    brows = [cpool.tile([1, FQ], BF16, tag=f"brow{i}", name=f"brow{i}")
             for i in range(3)]
    for i in range(3):
        nc.sync.dma_start(srows[i][:], sv_d[i:i + 1, :])
        nc.sync.dma_start(brows[i][:], bv_d[i:i + 1, :])

    with (
        tc.tile_pool(name="xt", bufs=1) as xtpool,
        tc.tile_pool(name="w", bufs=1) as wpool,
        tc.tile_pool(name="sq", bufs=2) as sqpool,
        tc.tile_pool(name="small", bufs=2) as smpool,
        tc.tile_pool(name="psA", bufs=1, space="PSUM") as psA,
        tc.tile_pool(name="psB", bufs=2, space="PSUM") as psB,
    ):
        xts = []
        for ci in range(NCT):
            xt_sb = xtpool.tile([128, T], BF16, tag=f"xt{ci}", name=f"xt{ci}")
            nc.sync.dma_start(xt_sb[:], xt_d[ci * 128:(ci + 1) * 128, :])
            xts.append(xt_sb)

        wqs, wks, wvs = [], [], []
        for ci in range(NCT):
            for lst, src, nm in ((wqs, wq_d, "wq"), (wks, wk_d, "wk"),
                                 (wvs, wv_d, "wv")):
                w_sb = wpool.tile([128, FQ], BF16, tag=f"{nm}{ci}",
                                  name=f"{nm}{ci}")
                nc.sync.dma_start(w_sb[:], src[ci * 128:(ci + 1) * 128, :])
                lst.append(w_sb)

        # ---- stats: sum(x) and sum(x^2) over c via ones-matmuls ----
        for tch in range(NQC):
            ts = slice(tch * QC, (tch + 1) * QC)
            ps_sum = psA.tile([1, QC], F32, tag="ps_sum")
            ps_ssq = psA.tile([1, QC], F32, tag="ps_ssq")
            for ci in range(NCT):
                nc.tensor.matmul(ps_sum[:], ones128[:], xts[ci][:, ts],
                                 start=(ci == 0), stop=(ci == NCT - 1))
                xsq = sqpool.tile([128, QC], BF16, tag="xsq")
                nc.vector.tensor_mul(xsq[:], xts[ci][:, ts], xts[ci][:, ts])
                nc.tensor.matmul(ps_ssq[:], ones128[:], xsq[:],
                                 start=(ci == 0), stop=(ci == NCT - 1))
            nc.scalar.activation(negmu[0:1, ts], ps_sum[:], AF.Copy,
                                 scale=-1.0 / C)
            mu2 = smpool.tile([1, QC], F32, tag="mu2")
            nc.vector.tensor_mul(mu2[:], negmu[0:1, ts], negmu[0:1, ts])
            mu2me = smpool.tile([1, QC], F32, tag="mu2me")
            nc.vector.tensor_scalar_add(mu2me[:], mu2[:], -EPS)
            vare = smpool.tile([1, QC], F32, tag="vare")
            nc.vector.scalar_tensor_tensor(vare[:], ps_ssq[:], 1.0 / C,
                                           mu2me[:], ALU.mult, ALU.subtract)
            nc.scalar.activation(recipa[0:1, ts], vare[:], AF.Sqrt)
            rvar = smpool.tile([1, QC], F32, tag="rvar")
            nc.vector.reciprocal(rvar[:], vare[:])
            a_row = smpool.tile([1, QC], F32, tag="a_row")
            nc.scalar.activation(a_row[:], rvar[:], AF.Sqrt)
            # broadcast A to 128 partitions
            ps_abc = psB.tile([128, QC], F32, tag="ps_v")
            nc.tensor.matmul(ps_abc[:], ones1[:], a_row[:],
                             start=True, stop=True)
            nc.scalar.activation(abc[:, ts], ps_abc[:], AF.Copy)
            nc.sync.dma_start(a_d[tch * QC:(tch + 1) * QC, 0:1], a_row[0:1, :])
        # A as per-t-tile columns via DRAM round-trip
        for ti in range(NTT):
            nc.sync.dma_start(a_col[:, ti:ti + 1],
                              a_d[ti * 128:(ti + 1) * 128, 0:1])

        # ---- Q^T / K^T f-tiles ----
        for p in range(2):
            for wlist, dst, srow, brow in (
                (wqs, q2t[p], srows[0], brows[0]),
                (wks, k2t[p], srows[1], brows[1]),
            ):
                fsl = slice(p * 128, (p + 1) * 128)
                for tch in range(NQC):
                    ts = slice(tch * QC, (tch + 1) * QC)
                    ps = psB.tile([128, QC], F32, tag="ps_qk")
                    for ci in range(NCT):
                        nc.tensor.matmul(ps[:], wlist[ci][:, fsl],
                                         xts[ci][:, ts], start=(ci == 0),
                                         stop=False)
                    nc.tensor.matmul(ps[:], srow[0:1, fsl], negmu[0:1, ts],
                                     start=False, stop=False)
                    nc.tensor.matmul(ps[:], brow[0:1, fsl], recipa[0:1, ts],
                                     start=False, stop=True)
                    nc.vector.tensor_mul(dst[:, ts], ps[:], abc[:, ts])

        # ---- V natural [t, f] with ones columns ----
        for ti in range(NTT):
            tsl = slice(ti * 128, (ti + 1) * 128)
            ps = psB.tile([128, NH, D], F32, tag="ps_v")
            ps2 = ps.rearrange("p a b -> p (a b)")
            for ci in range(NCT):
                nc.tensor.matmul(ps2, xts[ci][:, tsl], wvs[ci][:],
                                 start=(ci == 0), stop=False)
            nc.tensor.matmul(ps2, negmu[0:1, tsl], srows[2][:],
                             start=False, stop=False)
            nc.tensor.matmul(ps2, recipa[0:1, tsl], brows[2][:],
                             start=False, stop=True)
            nc.vector.tensor_scalar_mul(vna[ti][:, :, 0:D], ps[:],
                                        a_col[:, ti:ti + 1])
            nc.vector.memset(vna[ti][:, :, D:D + 1], 1.0)


def _emit_attention(tc, nc, q2t, k2t, vna, esel, attnu, denom, recips,
                    wo_d, out_d):
    QW = QC
    with (
        tc.tile_pool(name="exps", bufs=4) as epool,
        tc.tile_pool(name="spill", bufs=2) as spool,
        tc.tile_pool(name="wo2", bufs=1) as wo2pool,
        tc.tile_pool(name="osb", bufs=3) as opool,
        tc.tile_pool(name="ps_s", bufs=2, space="PSUM") as ps_s_pool,
        tc.tile_pool(name="ps_pv", bufs=1, space="PSUM") as ps_pv_pool,
        tc.tile_pool(name="ps_o", bufs=2, space="PSUM") as ps_o_pool,
    ):
        wos2 = []
        for p in range(2):
            wo_sb = wo2pool.tile([128, C], BF16, tag=f"wo2_{p}",
                                 name=f"wo2_{p}")
            nc.sync.dma_start(wo_sb[:], wo_d[p * 128:(p + 1) * 128, :])
            wos2.append(wo_sb)
        for qc in range(T // QW):
            qs = slice(qc * QW, (qc + 1) * QW)
            for p in range(2):
                ps_pv_a = ps_pv_pool.tile([D + 1, QW], F32, tag="pv_a")
                ps_pv_b = ps_pv_pool.tile([D + 1, QW], F32, tag="pv_b")
                for kt in range(NTT):
                    ksl = slice(kt * 128, (kt + 1) * 128)
                    ps_s2 = ps_s_pool.tile([128, 2 * QW], F32, tag="s2")
                    nc.tensor.matmul(ps_s2[:, 0:QW], k2t[p][0:D, ksl],
                                     q2t[p][0:D, qs], start=True, stop=True)
                    nc.tensor.matmul(ps_s2[:, QW:2 * QW], k2t[p][D:2 * D, ksl],
                                     q2t[p][D:2 * D, qs], start=True,
                                     stop=True)
                    es2 = epool.tile([128, 2 * QW], BF16, tag="es2")
                    nc.scalar.activation(es2[:], ps_s2[:], AF.Exp,
                                         scale=D ** -0.5)
                    nc.tensor.matmul(ps_pv_a[:], vna[kt][:, 2 * p, :],
                                     es2[:, 0:QW], start=(kt == 0),
                                     stop=(kt == NTT - 1))
                    nc.tensor.matmul(ps_pv_b[:], vna[kt][:, 2 * p + 1, :],
                                     es2[:, QW:2 * QW], start=(kt == 0),
                                     stop=(kt == NTT - 1))
                # evict: aligned DVE copies to SBUF, then SBUF->SBUF DMA for
                # the partition-shifted half and the denom rows
                nc.vector.tensor_copy(attnu[p][0:D, qs], ps_pv_a[0:D, :])
                tmpr = spool.tile([D + 1, QW], BF16, tag="tmpr")
                nc.vector.tensor_copy(tmpr[D:D + 1, :], ps_pv_a[D:D + 1, :])
                nc.sync.dma_start(denom[2 * p:2 * p + 1, qs],
                                  tmpr[D:D + 1, :])
                tmpb = spool.tile([D + 1, QW], BF16, tag="tmpb")
                nc.vector.tensor_copy(tmpb[:], ps_pv_b[:])
                nc.sync.dma_start(attnu[p][D:2 * D, qs], tmpb[0:D, :])
                nc.sync.dma_start(denom[2 * p + 1:2 * p + 2, qs],
                                  tmpb[D:D + 1, :])
            with nc.allow_low_precision(reason="f32r stores full fp32 bits"):
                nc.vector.reciprocal(recips[:, qs], denom[:, qs])
            for p in range(2):
                ps_r = ps_s_pool.tile([128, QW], F32, tag="s2")
                nc.tensor.matmul(ps_r[:], esel[:, p * 128:(p + 1) * 128],
                                 recips[:, qs], start=True, stop=True)
                nc.vector.tensor_mul(attnu[p][:, qs], attnu[p][:, qs],
                                     ps_r[:])
            for ti4 in range(QW // 128):
                ti = qc * (QW // 128) + ti4
                tsl = slice(ti * 128, (ti + 1) * 128)
                for oc in range(2):
                    osl = slice(oc * QC, (oc + 1) * QC)
                    ps_o = ps_o_pool.tile([128, QC], F32, tag="ps_o")
                    nc.tensor.matmul(ps_o[:], attnu[0][:, tsl],
                                     wos2[0][:, osl], start=True, stop=False)
                    nc.tensor.matmul(ps_o[:], attnu[1][:, tsl],
                                     wos2[1][:, osl], start=False, stop=True)
                    o_sb = opool.tile([128, QC], F32, tag="o_sb")
                    nc.vector.tensor_copy(o_sb[:], ps_o[:])
                    nc.sync.dma_start(out_d[tsl, osl], o_sb[:])


def _emit(tc):
    nc = tc.nc
    xt_d = nc.dram_tensor("xt", [C, T], BF16, kind="ExternalInput").ap()
    wq_d = nc.dram_tensor("wq", [C, FQ], BF16, kind="ExternalInput").ap()
    wk_d = nc.dram_tensor("wk", [C, FQ], BF16, kind="ExternalInput").ap()
    wv_d = nc.dram_tensor("wv", [C, FQ], BF16, kind="ExternalInput").ap()
    wo_d = nc.dram_tensor("wo", [FQ, C], BF16, kind="ExternalInput").ap()
    sv_d = nc.dram_tensor("svecs", [3, FQ], BF16, kind="ExternalInput").ap()
    bv_d = nc.dram_tensor("bvecs", [3, FQ], BF16, kind="ExternalInput").ap()
    es_d = nc.dram_tensor("esel", [NH, FQ], F32R, kind="ExternalInput").ap()
    out_d = nc.dram_tensor("out", [T, C], F32, kind="ExternalOutput").ap()
    a_d = nc.dram_tensor("a_scratch", [T, 1], F32, kind="Internal").ap()

    with (
        tc.tile_pool(name="const", bufs=1) as cpool,
        tc.tile_pool(name="persist", bufs=1) as ppool,
    ):
        esel = cpool.tile([NH, FQ], F32R, tag="esel")
        nc.sync.dma_start(esel[:], es_d[:])

        negmu = ppool.tile([1, T], BF16, tag="negmu")
        recipa = ppool.tile([1, T], BF16, tag="recipa")   # sqrt(var+eps) = 1/A
        abc = ppool.tile([128, T], F32R, tag="abc")       # A bcast to 128 parts
        a_col = ppool.tile([128, NTT], F32, tag="a_col")
        q2t = [ppool.tile([128, T], BF16, tag=f"q2t{p}", name=f"q2t{p}")
               for p in range(2)]
        k2t = [ppool.tile([128, T], BF16, tag=f"k2t{p}", name=f"k2t{p}")
               for p in range(2)]
        vna = [ppool.tile([128, NH, D + 1], BF16, tag=f"vna{i}",
                          name=f"vna{i}") for i in range(NTT)]

        _emit_proj(tc, nc, (xt_d, wq_d, wk_d, wv_d, sv_d, bv_d, a_d),
                   cpool, (negmu, recipa, abc, a_col, q2t, k2t, vna))

        with tc.tile_pool(name="late", bufs=1) as lpool:
            attnu = [lpool.tile([128, T], BF16, tag=f"attnu{p}",
                                name=f"attnu{p}") for p in range(2)]
            denom = lpool.tile([NH, T], BF16, tag="denom")
            recips = lpool.tile([NH, T], F32R, tag="recips")
            _emit_attention(tc, nc, q2t, k2t, vna, esel, attnu, denom,
                            recips, wo_d, out_d)


def _build():
    if "nc" in _CACHE:
        return _CACHE["nc"]
    nc = bacc.Bacc("TRN2", target_bir_lowering=False, debug=False,
                   enable_asserts=False)
    with tile.TileContext(nc) as tc:
        _emit(tc)
    nc.compile()
    _CACHE["nc"] = nc
    return nc


def kernel(x, gamma, beta, w_qkv, w_out, b_out):
    global LAST_RESULT
    x = np.asarray(x, np.float32)
    gamma = np.asarray(gamma, np.float32)
    beta = np.asarray(beta, np.float32)
    w_qkv = np.asarray(w_qkv, np.float32)
    w_out = np.asarray(w_out, np.float32)
    b_out = np.asarray(b_out, np.float32)

    nc = _build()

    wq_full = gamma[:, None] * w_qkv[:, 0:1024]
    wk_full = gamma[:, None] * w_qkv[:, 1024:2048]
    wv_full = gamma[:, None] * w_qkv[:, 2048:3072]
    bq_full = beta @ w_qkv[:, 0:1024]
    bk_full = beta @ w_qkv[:, 1024:2048]
    bv_full = beta @ w_qkv[:, 2048:3072]

    xts = [np.ascontiguousarray(x[b].T) for b in range(2)]
    esel = np.zeros((NH, FQ), np.float32)
    for h in range(NH):
        esel[h, h * D:(h + 1) * D] = 1.0

    in_maps = []
    for c in range(8):
        b, g = divmod(c, 4)
        fsl = slice(g * FQ, (g + 1) * FQ)
        wq = np.ascontiguousarray(wq_full[:, fsl])
        wk = np.ascontiguousarray(wk_full[:, fsl])
        wv = np.ascontiguousarray(wv_full[:, fsl])
        svecs = np.stack([wq.sum(0), wk.sum(0), wv.sum(0)])
        bvecs = np.stack([bq_full[fsl], bk_full[fsl], bv_full[fsl]])
        bf = ml_dtypes.bfloat16
        in_maps.append({
            "xt": xts[b].astype(bf),
            "wq": wq.astype(bf), "wk": wk.astype(bf), "wv": wv.astype(bf),
            "wo": np.ascontiguousarray(w_out[fsl, :]).astype(bf),
            "svecs": np.ascontiguousarray(svecs).astype(bf),
            "bvecs": np.ascontiguousarray(bvecs).astype(bf),
            "esel": esel,
        })

    trace = bool(int(os.environ.get("KERNEL_TRACE", "0")))
    trace_cores = None
    if trace:
        tc_env = os.environ.get("KERNEL_TRACE_CORES", "0")
        trace_cores = [int(v) for v in tc_env.split(",")]
    res = run_bass_kernel_spmd(nc, in_maps, core_ids=list(range(8)),
                               trace=trace, trace_cores=trace_cores)
    LAST_RESULT = res

    parts = [res.results[c]["out"] for c in range(8)]
    out = np.stack([
        parts[0] + parts[1] + parts[2] + parts[3],
        parts[4] + parts[5] + parts[6] + parts[7],
    ])
    return (out + b_out).astype(np.float32)
